# revision 14
# baseline (speedup 1.0000x reference)
"""PhysicsAttention (structured mesh 2D) Trainium2 kernel.

Data-parallel over batch: each of the 8 NeuronCores processes one batch
element end-to-end (no collectives).

Per-core pipeline (one batch element, mesh 128x128, N=16384 pixels),
pixel-major convolutions (x-window stationary, weights moving):
  phase A (per image row k of 128 px):
    logits conv : slice_w and 1/temperature folded into the conv_x
                  weights host-side -> one 3x3/128->512 conv. Fast
                  variant (folded bias == 0, the setup_inputs case):
                  fp8 x64 weights, 4 DoubleRow tap pairs + 1 plain
                  tap (row pitch 160 and shifted row copies keep every
                  slot step 16B-aligned). Fallback variant: bf16,
                  9 taps + K=1 bias matmul from an all-ones row.
    e=exp(/64)  : ACT exp straight out of PSUM (1/64 undoes fp8 scale)
    softmax     : per-head row sums (DVE reduce) + reciprocal ->
                  wT = e * (1/s)  (pixel-major, per-partition scale)
    conv_fx     : bf16, 9 tap matmuls, pixel-major
    slice_tok   : st[g,c] += wT.T @ [fx | 1]  (PSUM-resident accumulators)
    w_chm       : PE-transpose wT into channel-major Estore for phase C
  phase B (tiny): add norm*conv_fx_bias, normalize slice tokens, q/k/v,
    64-token attention, fold out_slice with out_w into M[g, d]
  phase C: outT[d, n] = sum_g M[g,:].T @ Estore[g, n]  (K=512, 4 blocks)

Host side: casts x to bf16 (shared by both convs), folds slice_w/temp
into the logits conv weights, appends the bias as a 10th weight row.
"""

import numpy as np
import ml_dtypes
from contextlib import ExitStack

B = 8
HM = WM = 128
DIM = 128
HEADS = 8
DH = 64
G = 64
INNER = 512
N = HM * WM
NCORES = 8
RT = 32  # row tiles (4 image rows each)

_CACHE = {}

def _build(use_bias):
    import concourse.bass as bass
    import concourse.tile as tile
    from concourse import bacc, mybir
    from concourse.masks import make_identity

    f32 = mybir.dt.float32
    bf16 = mybir.dt.bfloat16
    f8 = mybir.dt.float8e4
    AF = mybir.ActivationFunctionType
    AX = mybir.AxisListType
    DR = mybir.MatmulPerfMode.DoubleRow

    nc = bacc.Bacc("TRN2", target_bir_lowering=False, debug=False)
    # bf16 padded x, channel-major; row 130 is the bias lhsT pattern
    # (partition 0 = 1.0, partitions 1..127 = 0).
    xbp = nc.dram_tensor("xbp", [128, 131, 130], bf16, kind="ExternalInput").ap()
    if use_bias:
        wlg = nc.dram_tensor("wlg", [128, 10 * 512], bf16, kind="ExternalInput").ap()
    else:
        # fp8 copy of padded x at row pitch 160 (DoubleRow slot step % 16 == 0)
        x8p = nc.dram_tensor("x8p", [128, 131 * 160], f8, kind="ExternalInput").ap()
        wlg = nc.dram_tensor("wlg", [128, 9 * 512], f8, kind="ExternalInput").ap()
    wfx = nc.dram_tensor("wfx", [128, 9 * 512], bf16, kind="ExternalInput").ap()
    wqkv = nc.dram_tensor("wqkv", [128, 192], f32, kind="ExternalInput").ap()
    owt = nc.dram_tensor("owt", [64, 1024], f32, kind="ExternalInput").ap()
    bfxp = nc.dram_tensor("bfxp", [1, 512], f32, kind="ExternalInput").ap()
    outT = nc.dram_tensor("outT", [128, 16384], f32, kind="ExternalOutput").ap()

    with tile.TileContext(nc) as tc, ExitStack() as top:
        consts = top.enter_context(tc.tile_pool(name="consts", bufs=1))
        estP = top.enter_context(tc.tile_pool(name="estP", bufs=1))

        if use_bias:
            wlg_sb = consts.tile([128, 10, 512], bf16)
            nc.sync.dma_start(wlg_sb[:].rearrange("p a b -> p (a b)"), wlg[:])
        else:
            # rows 0-5: vertical pairs (0,kx)+(1,kx); rows 6-7: the
            # (2,1)+(2,2) pair; row 8: the lone (2,0) tap
            wlg_sb = consts.tile([128, 9, 512], f8)
            nc.sync.dma_start(wlg_sb[:].rearrange("p a b -> p (a b)"), wlg[:])
        wfx_sb = consts.tile([128, 9 * 512], bf16)
        nc.sync.dma_start(wfx_sb[:], wfx[:])
        wqkv_sb = consts.tile([128, 192], f32)
        nc.sync.dma_start(wqkv_sb[:], wqkv[:])
        owt_sb = consts.tile([64, 1024], f32)
        nc.sync.dma_start(owt_sb[:], owt[:])
        bfx_sb = consts.tile([1, 512], f32)
        nc.sync.dma_start(bfx_sb[:], bfxp[:])
        idbf = consts.tile([128, 128], bf16)
        make_identity(nc, idbf[:])
        idf32 = consts.tile([128, 128], f32)
        make_identity(nc, idf32[:])
        M_sb = consts.tile([128, 512], bf16)

        Estore = estP.tile([128, 4 * 16384], bf16)
        EstoreQ = Estore[:].rearrange("p (q n) -> p q n", q=4)

        with tc.tile_pool(name="stP", bufs=1, space="PSUM") as stP:
            psum_st0 = stP.tile([128, 258], f32, tag="st0")
            psum_st1 = stP.tile([128, 258], f32, tag="st1")
            st_banks = (psum_st0, psum_st1)

            with tc.tile_pool(name="xwin", bufs=2) as xwinP, \
                 tc.tile_pool(name="sbA", bufs=3) as sbA, \
                 tc.tile_pool(name="psA", bufs=2, space="PSUM") as psA:
                pend = None  # (wT, fx, gch) awaiting st-accum + transpose

                def flush_st(pend):
                    wT, fx, gch = pend
                    for p in range(4):
                        nc.tensor.matmul(
                            st_banks[p // 2][:, (p % 2) * 129:(p % 2) * 129 + 129],
                            wT[:, p * 128:(p + 1) * 128],
                            fx[:, p, :],
                            start=(gch == 0 and p % 2 == 0),
                            stop=(gch == 127 and p % 2 == 1))

                def flush_tr(pend):
                    wT, fx, gch = pend
                    pwc = psA.tile([128, 512], bf16, tag="pT", bufs=2)
                    for q in range(4):
                        nc.tensor.transpose(pwc[:, q * 128:(q + 1) * 128],
                                            wT[:, q * 128:(q + 1) * 128],
                                            idbf[:])
                    nc.scalar.activation(
                        EstoreQ[:, :, gch * 128:(gch + 1) * 128],
                        pwc[:].rearrange("p (q n) -> p q n", q=4), AF.Copy)

                for t in range(RT):
                    wb = xwinP.tile([128, 7, 130], bf16, tag="wb")
                    nc.sync.dma_start(wb[:, 0:6, :], xbp[:, 4 * t:4 * t + 6, :])
                    if use_bias:
                        nc.sync.dma_start(wb[:, 6, :], xbp[:, 130, :])
                    else:
                        w8 = xwinP.tile([128, 6, 160], f8, tag="w8")
                        nc.sync.dma_start(
                            w8[:].rearrange("p a b -> p (a b)"),
                            x8p[:, 4 * t * 160:(4 * t + 6) * 160])
                        # rows 4t+2..4t+5 shifted left by 1 (rows 0-3)
                        # and by 2 (rows 4-7)
                        w8s = xwinP.tile([128, 8, 160], f8, tag="w8s")
                        for sh in (1, 2):
                            nc.sync.dma_start(
                                w8s[:, 4 * (sh - 1):4 * sh, :].rearrange(
                                    "p a b -> p (a b)"),
                                x8p[:, (4 * t + 2) * 160 + sh:
                                       (4 * t + 6) * 160 + sh])
                    for k in range(4):
                        gch = 4 * t + k
                        pL = psA.tile([128, 512], f32, tag="pL", bufs=2)
                        if use_bias:
                            # --- bias (K=1) + 9 bf16 tap matmuls ---
                            nc.tensor.matmul(pL[:], wb[0:1, 6, 0:128],
                                             wlg_sb[0:1, 9, :],
                                             start=True, stop=False)
                            for tap in range(9):
                                ky, kx = tap // 3, tap % 3
                                nc.tensor.matmul(
                                    pL[:],
                                    wb[:, k + ky, kx:kx + 128],
                                    wlg_sb[:, tap, :],
                                    start=False, stop=(tap == 8))
                        else:
                            # --- 4 fp8 DoubleRow pairs + 1 plain tap ---
                            w8a, w8sa = w8[:], w8s[:]
                            for kx in range(3):
                                lhsT = bass.AP(
                                    tensor=w8a.tensor,
                                    offset=w8a.offset + k * 160 + kx,
                                    ap=[w8a.ap[0], [160, 2], [1, 128]])
                                nc.tensor.matmul(
                                    pL[:], lhsT,
                                    wlg_sb[:, 2 * kx:2 * kx + 2, :],
                                    start=(kx == 0), stop=False, perf_mode=DR)
                            lhsT = bass.AP(
                                tensor=w8sa.tensor,
                                offset=w8sa.offset + k * 160,
                                ap=[w8sa.ap[0], [4 * 160, 2], [1, 128]])
                            nc.tensor.matmul(pL[:], lhsT,
                                             wlg_sb[:, 6:8, :],
                                             start=False, stop=False,
                                             perf_mode=DR)
                            lhsT = bass.AP(
                                tensor=w8a.tensor,
                                offset=w8a.offset + (k + 2) * 160,
                                ap=[w8a.ap[0], [1, 128]])
                            nc.tensor.matmul(pL[:], lhsT,
                                             wlg_sb[:, 8, :],
                                             start=False, stop=True)
                        if pend is not None:
                            flush_st(pend)
                        e_pm = sbA.tile([128, 512], bf16, tag="e", bufs=3)
                        nc.scalar.activation(e_pm[:], pL[:], AF.Exp,
                                             scale=(1.0 if use_bias else 1.0 / 64.0))
                        s_k = sbA.tile([128, 8], f32, tag="s", bufs=3)
                        nc.vector.reduce_sum(
                            s_k[:], e_pm[:].rearrange("p (h g) -> p h g", h=8),
                            axis=AX.X)
                        r_k = sbA.tile([128, 8], f32, tag="r", bufs=3)
                        nc.vector.reciprocal(r_k[:], s_k[:])
                        wT = sbA.tile([128, 512], bf16, tag="wT", bufs=3)
                        r_b = bass.AP(tensor=r_k[:].tensor, offset=r_k[:].offset,
                                      ap=[r_k[:].ap[0], [1, 8], [0, 64]])
                        nc.vector.tensor_mul(wT[:], e_pm[:], r_b)
                        # --- value conv: 9 bf16 tap matmuls ---
                        pF = psA.tile([128, 512], f32, tag="pF", bufs=2)
                        for tap in range(9):
                            ky, kx = tap // 3, tap % 3
                            nc.tensor.matmul(
                                pF[:],
                                wb[:, k + ky, kx:kx + 128],
                                wfx_sb[:, tap * 512:(tap + 1) * 512],
                                start=(tap == 0), stop=(tap == 8))
                        if pend is not None:
                            flush_tr(pend)
                        fx = sbA.tile([128, 4, 129], bf16, tag="fx", bufs=3)
                        nc.scalar.activation(
                            fx[:, :, 0:128],
                            pF[:].rearrange("p (q n) -> p q n", q=4), AF.Copy)
                        nc.vector.memset(fx[:, :, 128:129], 1.0)
                        pend = (wT, fx, gch)
                flush_st(pend)
                flush_tr(pend)

            # ---- phase B ----
            with tc.tile_pool(name="sbB", bufs=2) as sbB, \
                 tc.tile_pool(name="psB", bufs=2, space="PSUM") as psB, \
                 tc.tile_pool(name="psM", bufs=1, space="PSUM") as psM:
                norm_c = sbB.tile([128, 4], f32, bufs=1)
                for b_ in range(2):
                    src = st_banks[b_][:]
                    nc.vector.tensor_copy(
                        norm_c[:, 2 * b_: 2 * b_ + 2],
                        bass.AP(tensor=src.tensor, offset=src.offset + 128,
                                ap=[src.ap[0], [129, 2]]))
                nflat = sbB.tile([1, 512], f32, bufs=1)
                for p in range(4):
                    pnT = psB.tile([128, 512], f32, tag="ptmp", bufs=1)
                    nc.tensor.transpose(pnT[0:1, 0:128], norm_c[:, p: p + 1], idf32[:])
                    nc.vector.tensor_copy(nflat[0:1, p * 128:(p + 1) * 128], pnT[0:1, 0:128])
                pbfx = psB.tile([128, 512], f32, tag="ptmp", bufs=1)
                for p in range(4):
                    nc.tensor.matmul(pbfx[:, p * 128:(p + 1) * 128],
                                     nflat[0:1, p * 128:(p + 1) * 128],
                                     bfx_sb[0:1, p * 128:(p + 1) * 128],
                                     start=(p == 0), stop=(p == 3))
                bfxo = sbB.tile([128, 512], f32, bufs=1)
                nc.vector.tensor_copy(bfxo[:], pbfx[:])
                ne = sbB.tile([128, 4], f32, bufs=1)
                nc.vector.tensor_scalar_add(ne[:], norm_c[:], 1e-5)
                rn = sbB.tile([128, 4], f32, bufs=1)
                nc.vector.reciprocal(rn[:], ne[:])
                stn = sbB.tile([128, 512], f32, bufs=1)
                for p in range(4):
                    nc.vector.tensor_add(
                        stn[:, p * 128:(p + 1) * 128],
                        st_banks[p // 2][:, (p % 2) * 129:(p % 2) * 129 + 128],
                        bfxo[:, p * 128:(p + 1) * 128])
                    nc.vector.tensor_scalar_mul(
                        stn[:, p * 128:(p + 1) * 128],
                        stn[:, p * 128:(p + 1) * 128],
                        rn[:, p: p + 1])
                pstnT = psB.tile([128, 512], f32, tag="ptmp", bufs=1)
                for p in range(4):
                    nc.tensor.transpose(pstnT[:, p * 128:(p + 1) * 128],
                                        stn[:, p * 128:(p + 1) * 128], idf32[:])
                stnT = sbB.tile([128, 512], f32, bufs=1)
                nc.vector.tensor_copy(stnT[:], pstnT[:])
                qkv_list = []
                for h in range(8):
                    p_, s_ = h // 2, h % 2
                    st_h = stnT[s_ * 64:(s_ + 1) * 64,
                                p_ * 128 + s_ * 64: p_ * 128 + s_ * 64 + 64]
                    wq_h = wqkv_sb[s_ * 64:(s_ + 1) * 64, :]
                    pq = psB.tile([64, 192], f32, tag="pqkv", bufs=1)
                    nc.tensor.matmul(pq[:, 0:64], wq_h[:, 0:64], st_h, start=True, stop=True)
                    nc.tensor.matmul(pq[:, 64:128], wq_h[:, 64:128], st_h, start=True, stop=True)
                    nc.tensor.matmul(pq[:, 128:192], st_h, wq_h[:, 128:192], start=True, stop=True)
                    qkv = sbB.tile([64, 192], f32, tag="qkv", bufs=8)
                    nc.vector.tensor_copy(qkv[:], pq[:])
                    qkv_list.append(qkv)
                pM = psM.tile([128, 512], f32)
                for h in range(8):
                    p_, s_ = h // 2, h % 2
                    qkv = qkv_list[h]
                    pa = psB.tile([64, 64], f32, tag="pa", bufs=3)
                    nc.tensor.matmul(pa[:], qkv[:, 0:64], qkv[:, 64:128], start=True, stop=True)
                    mx = sbB.tile([64, 1], f32, tag="mx")
                    nc.vector.reduce_max(mx[:], pa[:], axis=AX.X)
                    mxs = sbB.tile([64, 1], f32, tag="mxs")
                    nc.vector.tensor_scalar_mul(mxs[:], mx[:], -0.125)
                    aw = sbB.tile([64, 64], f32, tag="aw")
                    sa = sbB.tile([64, 1], f32, tag="sa")
                    nc.scalar.activation(aw[:], pa[:], AF.Exp,
                                         bias=mxs[:], scale=0.125, accum_out=sa[:])
                    rsa = sbB.tile([64, 1], f32, tag="rsa")
                    nc.vector.reciprocal(rsa[:], sa[:])
                    awn = sbB.tile([64, 64], f32, tag="awn")
                    nc.vector.tensor_scalar_mul(awn[:], aw[:], rsa[:])
                    paT = psB.tile([64, 64], f32, tag="pa", bufs=3)
                    nc.tensor.transpose(paT[:], awn[:], idf32[0:64, 0:64])
                    awT = sbB.tile([64, 64], f32, tag="awT")
                    nc.vector.tensor_copy(awT[:], paT[:])
                    poT = psB.tile([64, 64], f32, tag="pa", bufs=3)
                    nc.tensor.matmul(poT[:], qkv[:, 128:192], awT[:], start=True, stop=True)
                    oT = sbB.tile([64, 64], f32, tag="oT")
                    nc.vector.tensor_copy(oT[:], poT[:])
                    nc.tensor.matmul(pM[s_ * 64:(s_ + 1) * 64, p_ * 128:(p_ + 1) * 128],
                                     oT[:], owt_sb[:, h * 128:(h + 1) * 128],
                                     start=True, stop=True)
                nc.vector.tensor_copy(M_sb[:], pM[:])

        # ---- phase C ----
        with tc.tile_pool(name="sbC", bufs=3) as sbC, \
             tc.tile_pool(name="psC", bufs=3, space="PSUM") as psC:
            for i in range(32):
                po = psC.tile([128, 512], f32)
                for p in range(4):
                    nc.tensor.matmul(
                        po[:], M_sb[:, p * 128:(p + 1) * 128],
                        Estore[:, p * 16384 + i * 512: p * 16384 + (i + 1) * 512],
                        start=(p == 0), stop=(p == 3))
                ob = sbC.tile([128, 512], f32)
                nc.vector.tensor_copy(ob[:], po[:])
                nc.sync.dma_start(outT[:, i * 512:(i + 1) * 512], ob[:])

    nc.compile()
    return nc


def _prep(inputs):
    x = np.asarray(inputs["x"], dtype=np.float32)
    conv_fx_w = np.asarray(inputs["conv_fx_w"], dtype=np.float32)
    conv_fx_b = np.asarray(inputs["conv_fx_b"], dtype=np.float32)
    conv_x_w = np.asarray(inputs["conv_x_w"], dtype=np.float32)
    conv_x_b = np.asarray(inputs["conv_x_b"], dtype=np.float32)
    slice_w = np.asarray(inputs["slice_w"], dtype=np.float32)
    slice_b = np.asarray(inputs["slice_b"], dtype=np.float32)
    temperature = np.asarray(inputs["temperature"], dtype=np.float32)
    wq = np.asarray(inputs["wq"], dtype=np.float32)
    wk = np.asarray(inputs["wk"], dtype=np.float32)
    wv = np.asarray(inputs["wv"], dtype=np.float32)
    out_w = np.asarray(inputs["out_w"], dtype=np.float32)

    bf = ml_dtypes.bfloat16
    temp = np.clip(temperature.reshape(HEADS), 0.1, 5.0)

    # fold slice_w and 1/temp into conv_x -> logits conv
    Wlg = np.einsum("yxcHd,gd->yxcHg", conv_x_w.reshape(3, 3, DIM, HEADS, DH),
                    slice_w) / temp[None, None, None, :, None]
    Wlg = Wlg.reshape(3, 3, DIM, INNER)
    bias_lg = ((slice_b[None, :] + (slice_w @ conv_x_b.reshape(HEADS, DH).T).T)
               / temp[:, None]).reshape(INNER)
    use_bias = bool(np.any(bias_lg))
    if use_bias:
        # [cin, tap, cout]; row 9 partition 0 = the folded bias
        wlg_np = np.zeros((128, 10, 512), bf)
        wlg_np[:, 0:9, :] = Wlg.transpose(2, 0, 1, 3).reshape(
            128, 9, 512).astype(bf)
        wlg_np[0, 9, :] = bias_lg.astype(bf)
        wlg_np = np.ascontiguousarray(wlg_np.reshape(128, 10 * 512))
    else:
        f8 = ml_dtypes.float8_e4m3fn
        Wlg8 = (64.0 * Wlg).astype(f8)  # [ky, kx, cin, cout]
        wlg_np = np.zeros((128, 9, 512), f8)
        for kx in range(3):
            wlg_np[:, 2 * kx, :] = Wlg8[0, kx]      # DR pair slot 0
            wlg_np[:, 2 * kx + 1, :] = Wlg8[1, kx]  # DR pair slot 1
        wlg_np[:, 6, :] = Wlg8[2, 1]                # (2,1)+(2,2) pair
        wlg_np[:, 7, :] = Wlg8[2, 2]
        wlg_np[:, 8, :] = Wlg8[2, 0]                # lone tap
        wlg_np = np.ascontiguousarray(wlg_np.reshape(128, 9 * 512))

    wfx_np = np.ascontiguousarray(
        conv_fx_w.transpose(2, 0, 1, 3).reshape(128, 9 * 512)).astype(bf)

    wqkv_half = np.concatenate([wq.T, wk.T, wv.T], axis=1).astype(np.float32)
    wqkv_np = np.vstack([wqkv_half, wqkv_half])
    owt_np = np.ascontiguousarray(
        out_w.T.reshape(8, 64, 128).transpose(1, 0, 2).reshape(64, 1024))
    bfx_np = np.ascontiguousarray(conv_fx_b.reshape(1, 512))

    in_maps = []
    for b in range(B):
        xi = x[b].reshape(HM, WM, DIM).transpose(2, 0, 1)
        xb = np.zeros((128, 131, 130), bf)
        xb[:, 1:129, 1:129] = xi.astype(bf)
        xb[0, 130, :] = np.float32(1.0)  # bias lhsT row: partition 0 only
        m = {
            "xbp": xb, "wlg": wlg_np, "wfx": wfx_np,
            "wqkv": wqkv_np, "owt": owt_np, "bfxp": bfx_np,
        }
        if not use_bias:
            f8 = ml_dtypes.float8_e4m3fn
            x8 = np.zeros((128, 131, 160), f8)
            x8[:, 1:129, 1:129] = xi.astype(f8)
            m["x8p"] = x8.reshape(128, 131 * 160)
        in_maps.append(m)
    return in_maps, use_bias


def kernel(**inputs):
    from concourse.bass_utils import run_bass_kernel_spmd

    in_maps, use_bias = _prep(inputs)
    key = ("nc", use_bias)
    if key not in _CACHE:
        _CACHE[key] = _build(use_bias)
    nc = _CACHE["nc"] = _CACHE[key]
    res = run_bass_kernel_spmd(nc, in_maps, core_ids=list(range(NCORES)))
    out_b = np.asarray(inputs["out_b"], dtype=np.float32)
    out = np.empty((B, N, DIM), np.float32)
    for b in range(B):
        out[b] = res.results[b]["outT"].T + out_b
    return out


# revision 16
# speedup vs baseline: 1.0124x; 1.0124x over previous
"""PhysicsAttention (structured mesh 2D) Trainium2 kernel.

Data-parallel over batch: each of the 8 NeuronCores processes one batch
element end-to-end (no collectives).

Per-core pipeline (one batch element, mesh 128x128, N=16384 pixels),
pixel-major convolutions (x-window stationary, weights moving):
  phase A (per image row k of 128 px):
    logits conv : slice_w and 1/temperature folded into the conv_x
                  weights host-side -> one 3x3/128->512 conv. Fast
                  variant (folded bias == 0, the setup_inputs case):
                  fp8 x64 weights, 4 DoubleRow tap pairs + 1 plain
                  tap (row pitch 160 and shifted row copies keep every
                  slot step 16B-aligned). Fallback variant: bf16,
                  9 taps + K=1 bias matmul from an all-ones row.
    e=exp(/64)  : ACT exp straight out of PSUM (1/64 undoes fp8 scale)
    softmax     : per-head row sums (DVE reduce) + reciprocal ->
                  wT = e * (1/s)  (pixel-major, per-partition scale)
    conv_fx     : bf16, 9 tap matmuls, pixel-major
    slice_tok   : st[g,c] += wT.T @ [fx | 1]  (PSUM-resident accumulators)
    w_chm       : PE-transpose wT into channel-major Estore for phase C
  phase B (tiny): add norm*conv_fx_bias, normalize slice tokens, q/k/v,
    64-token attention, fold out_slice with out_w into M[g, d]
  phase C: outT[d, n] = sum_g M[g,:].T @ Estore[g, n]  (K=512, 4 blocks)

Host side: casts x to bf16 (shared by both convs), folds slice_w/temp
into the logits conv weights, appends the bias as a 10th weight row.
"""

import numpy as np
import ml_dtypes
from contextlib import ExitStack

B = 8
HM = WM = 128
DIM = 128
HEADS = 8
DH = 64
G = 64
INNER = 512
N = HM * WM
NCORES = 8
RT = 32  # row tiles (4 image rows each)

_CACHE = {}

def _build(use_bias):
    import concourse.bass as bass
    import concourse.tile as tile
    from concourse import bacc, mybir
    from concourse.masks import make_identity

    f32 = mybir.dt.float32
    bf16 = mybir.dt.bfloat16
    f8 = mybir.dt.float8e4
    AF = mybir.ActivationFunctionType
    AX = mybir.AxisListType
    DR = mybir.MatmulPerfMode.DoubleRow

    nc = bacc.Bacc("TRN2", target_bir_lowering=False, debug=False)
    # bf16 padded x, channel-major; row 130 is the bias lhsT pattern
    # (partition 0 = 1.0, partitions 1..127 = 0).
    xbp = nc.dram_tensor("xbp", [128, 131, 130], bf16, kind="ExternalInput").ap()
    if use_bias:
        wlg = nc.dram_tensor("wlg", [128, 10 * 512], bf16, kind="ExternalInput").ap()
    else:
        # fp8 copy of padded x at row pitch 160 (DoubleRow slot step % 16 == 0)
        x8p = nc.dram_tensor("x8p", [128, 131 * 160], f8, kind="ExternalInput").ap()
        wlg = nc.dram_tensor("wlg", [128, 9 * 512], f8, kind="ExternalInput").ap()
    wfx = nc.dram_tensor("wfx", [128, 9 * 512], bf16, kind="ExternalInput").ap()
    wqkv = nc.dram_tensor("wqkv", [128, 384], f32, kind="ExternalInput").ap()
    owt = nc.dram_tensor("owt", [128, 512], f32, kind="ExternalInput").ap()
    bfxp = nc.dram_tensor("bfxp", [1, 512], f32, kind="ExternalInput").ap()
    outT = nc.dram_tensor("outT", [128, 16384], f32, kind="ExternalOutput").ap()

    with tile.TileContext(nc) as tc, ExitStack() as top:
        consts = top.enter_context(tc.tile_pool(name="consts", bufs=1))
        estP = top.enter_context(tc.tile_pool(name="estP", bufs=1))

        if use_bias:
            wlg_sb = consts.tile([128, 10, 512], bf16)
            nc.sync.dma_start(wlg_sb[:].rearrange("p a b -> p (a b)"), wlg[:])
        else:
            # rows 0-5: vertical pairs (0,kx)+(1,kx); rows 6-7: the
            # (2,1)+(2,2) pair; row 8: the lone (2,0) tap
            wlg_sb = consts.tile([128, 9, 512], f8)
            nc.sync.dma_start(wlg_sb[:].rearrange("p a b -> p (a b)"), wlg[:])
        wfx_sb = consts.tile([128, 9 * 512], bf16)
        nc.sync.dma_start(wfx_sb[:], wfx[:])
        wqkv_sb = consts.tile([128, 384], f32)
        nc.sync.dma_start(wqkv_sb[:], wqkv[:])
        owt_sb = consts.tile([128, 512], f32)
        nc.sync.dma_start(owt_sb[:], owt[:])
        bfx_sb = consts.tile([1, 512], f32)
        nc.sync.dma_start(bfx_sb[:], bfxp[:])
        idbf = consts.tile([128, 128], bf16)
        make_identity(nc, idbf[:])
        idf32 = consts.tile([128, 128], f32)
        make_identity(nc, idf32[:])
        M_sb = consts.tile([128, 512], bf16)

        Estore = estP.tile([128, 4 * 16384], bf16)
        EstoreQ = Estore[:].rearrange("p (q n) -> p q n", q=4)

        with tc.tile_pool(name="stP", bufs=1, space="PSUM") as stP:
            psum_st0 = stP.tile([128, 258], f32, tag="st0")
            psum_st1 = stP.tile([128, 258], f32, tag="st1")
            st_banks = (psum_st0, psum_st1)

            with tc.tile_pool(name="xwin", bufs=2) as xwinP, \
                 tc.tile_pool(name="sbA", bufs=3) as sbA, \
                 tc.tile_pool(name="psA", bufs=2, space="PSUM") as psA:
                pend = None  # (wT, fx, gch) awaiting st-accum + transpose

                def flush_st(pend):
                    wT, fx, gch = pend
                    for p in range(4):
                        nc.tensor.matmul(
                            st_banks[p // 2][:, (p % 2) * 129:(p % 2) * 129 + 129],
                            wT[:, p * 128:(p + 1) * 128],
                            fx[:, p, :],
                            start=(gch == 0 and p % 2 == 0),
                            stop=(gch == 127 and p % 2 == 1))

                def flush_tr(pend):
                    wT, fx, gch = pend
                    pwc = psA.tile([128, 512], bf16, tag="pT", bufs=2)
                    for q in range(4):
                        nc.tensor.transpose(pwc[:, q * 128:(q + 1) * 128],
                                            wT[:, q * 128:(q + 1) * 128],
                                            idbf[:])
                    nc.scalar.activation(
                        EstoreQ[:, :, gch * 128:(gch + 1) * 128],
                        pwc[:].rearrange("p (q n) -> p q n", q=4), AF.Copy)

                for t in range(RT):
                    wb = xwinP.tile([128, 7, 130], bf16, tag="wb")
                    nc.sync.dma_start(wb[:, 0:6, :], xbp[:, 4 * t:4 * t + 6, :])
                    if use_bias:
                        nc.sync.dma_start(wb[:, 6, :], xbp[:, 130, :])
                    else:
                        w8 = xwinP.tile([128, 6, 160], f8, tag="w8")
                        nc.sync.dma_start(
                            w8[:].rearrange("p a b -> p (a b)"),
                            x8p[:, 4 * t * 160:(4 * t + 6) * 160])
                        # rows 4t+2..4t+5 shifted left by 1 (rows 0-3)
                        # and by 2 (rows 4-7)
                        w8s = xwinP.tile([128, 8, 160], f8, tag="w8s")
                        for sh in (1, 2):
                            nc.sync.dma_start(
                                w8s[:, 4 * (sh - 1):4 * sh, :].rearrange(
                                    "p a b -> p (a b)"),
                                x8p[:, (4 * t + 2) * 160 + sh:
                                       (4 * t + 6) * 160 + sh])
                    for k in range(4):
                        gch = 4 * t + k
                        pL = psA.tile([128, 512], f32, tag="pL", bufs=2)
                        if use_bias:
                            # --- bias (K=1) + 9 bf16 tap matmuls ---
                            nc.tensor.matmul(pL[:], wb[0:1, 6, 0:128],
                                             wlg_sb[0:1, 9, :],
                                             start=True, stop=False)
                            for tap in range(9):
                                ky, kx = tap // 3, tap % 3
                                nc.tensor.matmul(
                                    pL[:],
                                    wb[:, k + ky, kx:kx + 128],
                                    wlg_sb[:, tap, :],
                                    start=False, stop=(tap == 8))
                        else:
                            # --- 4 fp8 DoubleRow pairs + 1 plain tap ---
                            w8a, w8sa = w8[:], w8s[:]
                            for kx in range(3):
                                lhsT = bass.AP(
                                    tensor=w8a.tensor,
                                    offset=w8a.offset + k * 160 + kx,
                                    ap=[w8a.ap[0], [160, 2], [1, 128]])
                                nc.tensor.matmul(
                                    pL[:], lhsT,
                                    wlg_sb[:, 2 * kx:2 * kx + 2, :],
                                    start=(kx == 0), stop=False, perf_mode=DR)
                            lhsT = bass.AP(
                                tensor=w8sa.tensor,
                                offset=w8sa.offset + k * 160,
                                ap=[w8sa.ap[0], [4 * 160, 2], [1, 128]])
                            nc.tensor.matmul(pL[:], lhsT,
                                             wlg_sb[:, 6:8, :],
                                             start=False, stop=False,
                                             perf_mode=DR)
                            lhsT = bass.AP(
                                tensor=w8a.tensor,
                                offset=w8a.offset + (k + 2) * 160,
                                ap=[w8a.ap[0], [1, 128]])
                            nc.tensor.matmul(pL[:], lhsT,
                                             wlg_sb[:, 8, :],
                                             start=False, stop=True)
                        if pend is not None:
                            flush_st(pend)
                        e_pm = sbA.tile([128, 512], bf16, tag="e", bufs=3)
                        nc.scalar.activation(e_pm[:], pL[:], AF.Exp,
                                             scale=(1.0 if use_bias else 1.0 / 64.0))
                        s_k = sbA.tile([128, 8], f32, tag="s", bufs=3)
                        nc.vector.reduce_sum(
                            s_k[:], e_pm[:].rearrange("p (h g) -> p h g", h=8),
                            axis=AX.X)
                        r_k = sbA.tile([128, 8], f32, tag="r", bufs=3)
                        nc.vector.reciprocal(r_k[:], s_k[:])
                        wT = sbA.tile([128, 512], bf16, tag="wT", bufs=3)
                        r_b = bass.AP(tensor=r_k[:].tensor, offset=r_k[:].offset,
                                      ap=[r_k[:].ap[0], [1, 8], [0, 64]])
                        nc.vector.tensor_mul(wT[:], e_pm[:], r_b)
                        # --- value conv: 9 bf16 tap matmuls ---
                        pF = psA.tile([128, 512], f32, tag="pF", bufs=2)
                        for tap in range(9):
                            ky, kx = tap // 3, tap % 3
                            nc.tensor.matmul(
                                pF[:],
                                wb[:, k + ky, kx:kx + 128],
                                wfx_sb[:, tap * 512:(tap + 1) * 512],
                                start=(tap == 0), stop=(tap == 8))
                        if pend is not None:
                            flush_tr(pend)
                        fx = sbA.tile([128, 4, 129], bf16, tag="fx", bufs=3)
                        nc.scalar.activation(
                            fx[:, :, 0:128],
                            pF[:].rearrange("p (q n) -> p q n", q=4), AF.Copy)
                        nc.vector.memset(fx[:, :, 128:129], 1.0)
                        pend = (wT, fx, gch)
                flush_st(pend)
                flush_tr(pend)

            # ---- phase B ----
            with tc.tile_pool(name="sbB", bufs=2) as sbB, \
                 tc.tile_pool(name="psB", bufs=2, space="PSUM") as psB, \
                 tc.tile_pool(name="psM", bufs=1, space="PSUM") as psM:
                norm_c = sbB.tile([128, 4], f32, bufs=1)
                for b_ in range(2):
                    src = st_banks[b_][:]
                    nc.vector.tensor_copy(
                        norm_c[:, 2 * b_: 2 * b_ + 2],
                        bass.AP(tensor=src.tensor, offset=src.offset + 128,
                                ap=[src.ap[0], [129, 2]]))
                nflat = sbB.tile([1, 512], f32, bufs=1)
                for p in range(4):
                    pnT = psB.tile([128, 512], f32, tag="ptmp", bufs=1)
                    nc.tensor.transpose(pnT[0:1, 0:128], norm_c[:, p: p + 1], idf32[:])
                    nc.vector.tensor_copy(nflat[0:1, p * 128:(p + 1) * 128], pnT[0:1, 0:128])
                pbfx = psB.tile([128, 512], f32, tag="ptmp", bufs=1)
                for p in range(4):
                    nc.tensor.matmul(pbfx[:, p * 128:(p + 1) * 128],
                                     nflat[0:1, p * 128:(p + 1) * 128],
                                     bfx_sb[0:1, p * 128:(p + 1) * 128],
                                     start=(p == 0), stop=(p == 3))
                bfxo = sbB.tile([128, 512], f32, bufs=1)
                nc.vector.tensor_copy(bfxo[:], pbfx[:])
                ne = sbB.tile([128, 4], f32, bufs=1)
                nc.vector.tensor_scalar_add(ne[:], norm_c[:], 1e-5)
                rn = sbB.tile([128, 4], f32, bufs=1)
                nc.vector.reciprocal(rn[:], ne[:])
                # block-diagonal-clean slice tokens: only each head's own
                # (g, c) block is populated; cross-head blocks stay zero
                stn = sbB.tile([128, 512], f32, bufs=1)
                nc.vector.memset(stn[:], 0.0)
                for p in range(4):
                    for j in range(2):
                        js = slice(j * 64, j * 64 + 64)
                        cs = slice(p * 128 + j * 64, p * 128 + j * 64 + 64)
                        nc.vector.tensor_add(
                            stn[js, cs],
                            st_banks[p // 2][js, (p % 2) * 129 + j * 64:
                                             (p % 2) * 129 + j * 64 + 64],
                            bfxo[js, cs])
                    nc.scalar.activation(stn[:, p * 128:(p + 1) * 128],
                                         stn[:, p * 128:(p + 1) * 128],
                                         AF.Copy, scale=rn[:, p: p + 1])
                pstnT = psB.tile([128, 512], f32, tag="ptmp", bufs=1)
                for p in range(4):
                    nc.tensor.transpose(pstnT[:, p * 128:(p + 1) * 128],
                                        stn[:, p * 128:(p + 1) * 128], idf32[:])
                stnT = sbB.tile([128, 512], f32, bufs=1)
                nc.vector.tensor_copy(stnT[:], pstnT[:])
                # q/k/v for a head pair in one go via block-diagonal weights
                qall_list = []
                for p_ in range(4):
                    stp = stnT[:, p_ * 128:(p_ + 1) * 128]
                    pq2 = psB.tile([128, 384], f32, tag="pqkv", bufs=1)
                    nc.tensor.matmul(pq2[:, 0:128], wqkv_sb[:, 0:128], stp,
                                     start=True, stop=True)
                    nc.tensor.matmul(pq2[:, 128:256], wqkv_sb[:, 128:256], stp,
                                     start=True, stop=True)
                    nc.tensor.matmul(pq2[:, 256:384], stp, wqkv_sb[:, 256:384],
                                     start=True, stop=True)
                    qa = sbB.tile([128, 384], f32, tag="qkv", bufs=4)
                    nc.vector.tensor_copy(qa[:], pq2[:])
                    qall_list.append(qa)
                pM = psM.tile([128, 512], f32)
                for p_ in range(4):
                    qa = qall_list[p_]
                    pa2 = psB.tile([128, 128], f32, tag="pa", bufs=3)
                    nc.tensor.matmul(pa2[:], qa[:, 0:128], qa[:, 128:256],
                                     start=True, stop=True)
                    mx = sbB.tile([128, 1], f32, tag="mx")
                    sa = sbB.tile([128, 1], f32, tag="sa")
                    for j in range(2):
                        js = slice(j * 64, j * 64 + 64)
                        nc.vector.reduce_max(mx[js, :],
                                             pa2[js, j * 64:j * 64 + 64],
                                             axis=AX.X)
                    mxs = sbB.tile([128, 1], f32, tag="mxs")
                    nc.vector.tensor_scalar_mul(mxs[:], mx[:], -0.125)
                    aw2 = sbB.tile([128, 128], f32, tag="aw")
                    nc.scalar.activation(aw2[:], pa2[:], AF.Exp,
                                         bias=mxs[:], scale=0.125)
                    for j in range(2):
                        js = slice(j * 64, j * 64 + 64)
                        nc.vector.reduce_sum(sa[js, :],
                                             aw2[js, j * 64:j * 64 + 64],
                                             axis=AX.X)
                    rsa = sbB.tile([128, 1], f32, tag="rsa")
                    nc.vector.reciprocal(rsa[:], sa[:])
                    awn2 = sbB.tile([128, 128], f32, tag="awn")
                    nc.vector.tensor_scalar_mul(awn2[:], aw2[:], rsa[:])
                    nc.vector.memset(awn2[0:64, 64:128], 0.0)
                    nc.vector.memset(awn2[64:128, 0:64], 0.0)
                    paT2 = psB.tile([128, 128], f32, tag="pa", bufs=3)
                    nc.tensor.transpose(paT2[:], awn2[:], idf32[:])
                    awT2 = sbB.tile([128, 128], f32, tag="awT")
                    nc.vector.tensor_copy(awT2[:], paT2[:])
                    poT2 = psB.tile([128, 128], f32, tag="pa", bufs=3)
                    nc.tensor.matmul(poT2[:], qa[:, 256:384], awT2[:],
                                     start=True, stop=True)
                    oT2 = sbB.tile([128, 128], f32, tag="oT")
                    nc.vector.tensor_copy(oT2[:], poT2[:])
                    nc.tensor.matmul(pM[:, p_ * 128:(p_ + 1) * 128],
                                     oT2[:], owt_sb[:, p_ * 128:(p_ + 1) * 128],
                                     start=True, stop=True)
                nc.vector.tensor_copy(M_sb[:], pM[:])

        # ---- phase C ----
        with tc.tile_pool(name="sbC", bufs=3) as sbC, \
             tc.tile_pool(name="psC", bufs=3, space="PSUM") as psC:
            for i in range(32):
                po = psC.tile([128, 512], f32)
                for p in range(4):
                    nc.tensor.matmul(
                        po[:], M_sb[:, p * 128:(p + 1) * 128],
                        Estore[:, p * 16384 + i * 512: p * 16384 + (i + 1) * 512],
                        start=(p == 0), stop=(p == 3))
                ob = sbC.tile([128, 512], f32)
                nc.vector.tensor_copy(ob[:], po[:])
                nc.sync.dma_start(outT[:, i * 512:(i + 1) * 512], ob[:])

    nc.compile()
    return nc


def _prep(inputs):
    x = np.asarray(inputs["x"], dtype=np.float32)
    conv_fx_w = np.asarray(inputs["conv_fx_w"], dtype=np.float32)
    conv_fx_b = np.asarray(inputs["conv_fx_b"], dtype=np.float32)
    conv_x_w = np.asarray(inputs["conv_x_w"], dtype=np.float32)
    conv_x_b = np.asarray(inputs["conv_x_b"], dtype=np.float32)
    slice_w = np.asarray(inputs["slice_w"], dtype=np.float32)
    slice_b = np.asarray(inputs["slice_b"], dtype=np.float32)
    temperature = np.asarray(inputs["temperature"], dtype=np.float32)
    wq = np.asarray(inputs["wq"], dtype=np.float32)
    wk = np.asarray(inputs["wk"], dtype=np.float32)
    wv = np.asarray(inputs["wv"], dtype=np.float32)
    out_w = np.asarray(inputs["out_w"], dtype=np.float32)

    bf = ml_dtypes.bfloat16
    temp = np.clip(temperature.reshape(HEADS), 0.1, 5.0)

    # fold slice_w and 1/temp into conv_x -> logits conv
    Wlg = np.einsum("yxcHd,gd->yxcHg", conv_x_w.reshape(3, 3, DIM, HEADS, DH),
                    slice_w) / temp[None, None, None, :, None]
    Wlg = Wlg.reshape(3, 3, DIM, INNER)
    bias_lg = ((slice_b[None, :] + (slice_w @ conv_x_b.reshape(HEADS, DH).T).T)
               / temp[:, None]).reshape(INNER)
    use_bias = bool(np.any(bias_lg))
    if use_bias:
        # [cin, tap, cout]; row 9 partition 0 = the folded bias
        wlg_np = np.zeros((128, 10, 512), bf)
        wlg_np[:, 0:9, :] = Wlg.transpose(2, 0, 1, 3).reshape(
            128, 9, 512).astype(bf)
        wlg_np[0, 9, :] = bias_lg.astype(bf)
        wlg_np = np.ascontiguousarray(wlg_np.reshape(128, 10 * 512))
    else:
        f8 = ml_dtypes.float8_e4m3fn
        Wlg8 = (64.0 * Wlg).astype(f8)  # [ky, kx, cin, cout]
        wlg_np = np.zeros((128, 9, 512), f8)
        for kx in range(3):
            wlg_np[:, 2 * kx, :] = Wlg8[0, kx]      # DR pair slot 0
            wlg_np[:, 2 * kx + 1, :] = Wlg8[1, kx]  # DR pair slot 1
        wlg_np[:, 6, :] = Wlg8[2, 1]                # (2,1)+(2,2) pair
        wlg_np[:, 7, :] = Wlg8[2, 2]
        wlg_np[:, 8, :] = Wlg8[2, 0]                # lone tap
        wlg_np = np.ascontiguousarray(wlg_np.reshape(128, 9 * 512))

    wfx_np = np.ascontiguousarray(
        conv_fx_w.transpose(2, 0, 1, 3).reshape(128, 9 * 512)).astype(bf)

    wqkv_np = np.zeros((128, 384), np.float32)
    for j in range(2):
        blk = slice(j * 64, j * 64 + 64)
        wqkv_np[blk, 0:128][:, blk] = wq.T
        wqkv_np[blk, 128:256][:, blk] = wk.T
        wqkv_np[blk, 256:384][:, blk] = wv.T
    owt_np = np.zeros((128, 512), np.float32)
    for h in range(8):
        p_, j = h // 2, h % 2
        owt_np[j * 64:(j + 1) * 64, p_ * 128:(p_ + 1) * 128] = \
            out_w[:, h * 64:(h + 1) * 64].T
    bfx_np = np.ascontiguousarray(conv_fx_b.reshape(1, 512))

    in_maps = []
    for b in range(B):
        xi = x[b].reshape(HM, WM, DIM).transpose(2, 0, 1)
        xb = np.zeros((128, 131, 130), bf)
        xb[:, 1:129, 1:129] = xi.astype(bf)
        xb[0, 130, :] = np.float32(1.0)  # bias lhsT row: partition 0 only
        m = {
            "xbp": xb, "wlg": wlg_np, "wfx": wfx_np,
            "wqkv": wqkv_np, "owt": owt_np, "bfxp": bfx_np,
        }
        if not use_bias:
            f8 = ml_dtypes.float8_e4m3fn
            x8 = np.zeros((128, 131, 160), f8)
            x8[:, 1:129, 1:129] = xi.astype(f8)
            m["x8p"] = x8.reshape(128, 131 * 160)
        in_maps.append(m)
    return in_maps, use_bias


def kernel(**inputs):
    from concourse.bass_utils import run_bass_kernel_spmd

    in_maps, use_bias = _prep(inputs)
    key = ("nc", use_bias)
    if key not in _CACHE:
        _CACHE[key] = _build(use_bias)
    nc = _CACHE["nc"] = _CACHE[key]
    res = run_bass_kernel_spmd(nc, in_maps, core_ids=list(range(NCORES)))
    out_b = np.asarray(inputs["out_b"], dtype=np.float32)
    out = np.empty((B, N, DIM), np.float32)
    for b in range(B):
        out[b] = res.results[b]["outT"].T + out_b
    return out
